# revision 29
# baseline (speedup 1.0000x reference)
"""Trainium2 Bass kernel for CachedMultiheadAttention (sliding-window + ALiBi).

Sharding: 8 cores = 2 batches x 4 head-quartets. Core c handles batch c//4 and
heads [4*(c%4), 4*(c%4)+4). Each core computes QKV projection for its heads,
banded attention (causal + 512 window + ALiBi), and a partial out-projection
over its heads' 256 embedding columns. Host sums the 4 partials per batch.

v2: all-bf16 dataflow, software-pipelined phases (QKV / attention / out-proj
interleaved so PE never drains), V produced directly in natural [t, d] layout
(x stationary, wv streaming; no PE transposes), single-instruction 640-wide
exp from a 2-bank PSUM tile, P*ebias multiplies on gpsimd, fast approximate
reciprocal for softmax denominators. Engine budget per core ~: PE 80us,
ACT ~60us, DVE ~60us, gpsimd ~50us.
"""
import math

import numpy as np
import ml_dtypes

import concourse.bass as bass
import concourse.tile as tile
from concourse import bacc, mybir
from concourse.bass_utils import run_bass_kernel_spmd

F32 = mybir.dt.float32
BF16 = mybir.dt.bfloat16

B, T, E, H, HD, W = 2, 2048, 1024, 16, 64, 512
NCORES = 8
HL = 4                # local heads per core
NT = T // 128         # 16 key blocks
QW = 640              # q-window per key block (128 + 512 sliding window)

_CACHE = {}


def _get_slopes(n):
    def p2(m):
        start = 2 ** (-(2 ** (-(math.log2(m) - 3))))
        return [start * start**i for i in range(m)]
    if math.log2(n) % 1 == 0:
        return p2(n)
    c = 2 ** math.floor(math.log2(n))
    return p2(c) + _get_slopes(2 * c)[0::2][: n - c]


def _build(dbg=False):
    nc = bacc.Bacc("TRN2", target_bir_lowering=False, debug=False, num_devices=NCORES)
    xT = nc.dram_tensor("xT", [4, 8, 128, 512], BF16, kind="ExternalInput").ap()
    wqkv = nc.dram_tensor("wqkv", [8, 128, 768], BF16, kind="ExternalInput").ap()
    wo = nc.dram_tensor("wo", [2, 128, E], BF16, kind="ExternalInput").ap()
    biasd = nc.dram_tensor("biasd", [HL, 128, QW], BF16, kind="ExternalInput").ap()
    outT = nc.dram_tensor("outT", [8, 128, T], BF16, kind="ExternalOutput").ap()
    if dbg:
        d_qkvT = nc.dram_tensor("d_qkvT", [128, 4, T], BF16, kind="ExternalOutput").ap()
        d_vnat = nc.dram_tensor("d_vnat", [128, NT, HL, HD + 1], BF16, kind="ExternalOutput").ap()
        d_pth = nc.dram_tensor("d_pth", [128, 4, NT, QW], BF16, kind="ExternalOutput").ap()
        d_ao2T = nc.dram_tensor("d_ao2T", [128, 2, T], BF16, kind="ExternalOutput").ap()
        d_aoraw = nc.dram_tensor("d_aoraw", [4, 65, 512], F32, kind="ExternalOutput").ap()
        d_rec = nc.dram_tensor("d_rec", [4, 1, 512], F32, kind="ExternalOutput").ap()
        d_bc = nc.dram_tensor("d_bc", [4, 64, 512], F32, kind="ExternalOutput").ap()

    with tile.TileContext(nc) as tc:
        with (
            tc.tile_pool(name="singles", bufs=1) as singles,
            tc.tile_pool(name="xp", bufs=3) as xp,
            tc.tile_pool(name="prawp", bufs=3) as prawp,
            tc.tile_pool(name="smallp", bufs=3) as smallp,
            tc.tile_pool(name="evp", bufs=4) as evp,
            tc.tile_pool(name="mm", bufs=2, space="PSUM") as mmp,
            tc.tile_pool(name="av", bufs=2, space="PSUM") as avp,
            tc.tile_pool(name="sp", bufs=2, space="PSUM") as spp,
        ):
            wqkv_sb = singles.tile([128, 8, 768], BF16)
            wo_sb = singles.tile([128, 2, E], BF16)
            bias_sb = singles.tile([128, HL, QW], BF16)
            qkvT = singles.tile([128, 4, T], BF16)   # slots: Qp0 Qp1 Kp0 Kp1
            vnat = singles.tile([128, NT, HL, HD + 1], BF16)
            ao2T = singles.tile([128, 2, T], BF16)   # normalized AO^T (bf16)

            nc.gpsimd.memset(vnat[:], 1.0)           # ones col at [..., 64]

            xcs = {}
            xT_r = xT.rearrange("b c p t -> b p c t")

            def load_x(tb):
                xc = xp.tile([128, 8, 512], BF16, tag="xc")
                xcs[tb] = xc
                for ec in range(8):
                    eng = nc.sync if ec % 2 == 0 else nc.scalar
                    eng.dma_start(xc[:, ec, :], xT[tb, ec])

            # PE warmup: dummy matmuls on zeros keep HAM busy while the
            # initial DMAs land, so real QKV work starts at full clock
            wup = singles.tile([128, 640], BF16)
            nc.vector.memset(wup[:], 0.0)
            wup_ps = mmp.tile([128, 512], F32, tag="mm512")
            for _ in range(24):
                nc.tensor.matmul(
                    wup_ps[:], lhsT=wup[:, 0:128], rhs=wup[:, 0:512],
                    start=True, stop=True,
                )

            # loads in descending priority on the two HWDGE queues: the queue
            # order is the DGE service order, so first-needed bytes go first.
            xc0 = xp.tile([128, 8, 512], BF16, tag="xc")
            xcs[0] = xc0
            wq_r = wqkv.rearrange("c p m -> p c m")
            for ec in range(8):
                e1, e2 = (nc.sync, nc.scalar) if ec % 2 == 0 else (nc.scalar, nc.sync)
                e1.dma_start(xc0[:, ec, :], xT[0, ec])
                e2.dma_start(wqkv_sb[:, ec, 0:512], wq_r[:, ec, 0:512])
            for ec in range(8):
                eng = nc.sync if ec % 2 == 0 else nc.scalar
                eng.dma_start(wqkv_sb[:, ec, 512:768], wq_r[:, ec, 512:768])
            nc.sync.dma_start(bias_sb[:], biasd.rearrange("h p c -> p h c"))
            nc.scalar.dma_start(wo_sb[:], wo.rearrange("c p f -> p c f"))

            def qkv_group(tb, m):
                # Q/K projection chunk m (0..3): transposed layout [d_local, t]
                xc = xcs[tb]
                pt = mmp.tile([128, 512], F32, tag="mm512")
                for ec in range(8):
                    nc.tensor.matmul(
                        pt[:],
                        lhsT=wqkv_sb[:, ec, m * 128:(m + 1) * 128],
                        rhs=xc[:, ec, :],
                        start=(ec == 0), stop=(ec == 7),
                    )
                nc.vector.tensor_copy(qkvT[:, m, tb * 512:(tb + 1) * 512], pt[:])

            def v_group(tb, half):
                # V in natural layout: x chunk stationary, wv streaming.
                # Two t-chunks share one PSUM tile / one evac.
                xc = xcs[tb]
                tc0 = tb * 4 + 2 * half
                pv = mmp.tile([128, 512], F32, tag="mm512")
                for sub in range(2):
                    tcq = 2 * half + sub
                    for ec in range(8):
                        nc.tensor.matmul(
                            pv[:, sub * 256:(sub + 1) * 256],
                            lhsT=xc[:, ec, tcq * 128:(tcq + 1) * 128],
                            rhs=wqkv_sb[:, ec, 512:768],
                            start=(ec == 0), stop=(ec == 7),
                            skip_group_check=True,
                        )
                nc.scalar.copy(
                    vnat[:, tc0:tc0 + 2, 0:4, 0:HD],
                    pv[:].rearrange("p (s h d) -> p s h d", s=2, h=4),
                )

            def s_pair(sq, jb):
                # S^T[j, q] for both heads of pair sq, q in [jb*128, jb*128+qw).
                # hh=0 on PE rows 0:64, hh=1 on rows 64:128 -> the two K=64
                # matmul streams run concurrently on the array.
                qw = min(QW, T - jb * 128)
                w0 = min(qw, 512)
                sts = []
                for hh in range(2):
                    r0 = hh * 64
                    st = spp.tile([128, QW], F32, tag="s640")
                    sts.append(st)
                    nc.tensor.matmul(
                        st[:, 0:w0],
                        lhsT=qkvT[r0:r0 + 64, 2 + sq, jb * 128:(jb + 1) * 128],
                        rhs=qkvT[r0:r0 + 64, sq, jb * 128:jb * 128 + w0],
                        start=True, stop=True,
                    )
                for hh in range(2):
                    if qw > 512:
                        r0 = hh * 64
                        nc.tensor.matmul(
                            sts[hh][:, 512:qw],
                            lhsT=qkvT[r0:r0 + 64, 2 + sq, jb * 128:(jb + 1) * 128],
                            rhs=qkvT[r0:r0 + 64, sq, jb * 128 + 512:jb * 128 + qw],
                            start=True, stop=True,
                        )
                for hh in range(2):
                    h = 2 * sq + hh
                    praw = prawp.tile([128, QW], BF16, tag="praw")
                    nc.scalar.activation(
                        out=praw[:, 0:qw], in_=sts[hh][:, 0:qw],
                        func=mybir.ActivationFunctionType.Exp,
                    )
                    nc.vector.tensor_tensor(
                        out=pth[:, h, jb, 0:qw], in0=praw[:, 0:qw],
                        in1=bias_sb[:, h, 0:qw], op=mybir.AluOpType.mult,
                    )

            def av_block(sq, hh, g):
                h = 2 * sq + hh
                r0 = hh * 64
                ao = avp.tile([128, 512], F32, tag="ao")
                jbs = [4 * g] + [jb for jb in range(max(0, 4 * g - 4), 4 * g + 4)
                                 if jb != 4 * g]
                for i, jb in enumerate(jbs):
                    qb_lo = max(4 * g, jb)
                    qb_hi = min(4 * g + 3, jb + 4)
                    wdt = (qb_hi - qb_lo + 1) * 128
                    ao_off = (qb_lo - 4 * g) * 128
                    p_off = (qb_lo - jb) * 128
                    nc.tensor.matmul(
                        ao[0:65, ao_off:ao_off + wdt],
                        lhsT=vnat[:, jb, hh + 2 * sq, :],
                        rhs=pth[:, 2 * sq + hh, jb, p_off:p_off + wdt],
                        start=(i == 0), stop=(i == len(jbs) - 1),
                        skip_group_check=True,
                    )
                rs0 = smallp.tile([1, 512], F32, tag="rs0")
                nc.scalar.copy(rs0[:], ao[64:65, :])
                rec = smallp.tile([1, 512], F32, tag="rec")
                nc.vector.reciprocal_approx_fast(rec[:], rs0[:])
                bc = smallp.tile([64, 512], F32, tag="bc")
                nc.gpsimd.partition_broadcast(bc[:], rec[:])
                if dbg and g == 0:
                    di = 2 * sq + hh
                    draw = evp.tile([65, 512], F32, tag="draw")
                    nc.vector.tensor_copy(draw[:], ao[0:65, :])
                    nc.sync.dma_start(d_aoraw[di], draw[:])
                    nc.sync.dma_start(d_rec[di], rec[:])
                    nc.sync.dma_start(d_bc[di], bc[:])
                nc.vector.tensor_tensor(
                    out=ao2T[r0:r0 + 64, sq, g * 512:(g + 1) * 512],
                    in0=ao[0:64, :], in1=bc[:], op=mybir.AluOpType.mult,
                )

            def op_group(tb, fc):
                po = mmp.tile([128, 512], F32, tag="mm512")
                for c2 in range(2):
                    nc.tensor.matmul(
                        po[:],
                        lhsT=wo_sb[:, c2, fc * 128:(fc + 1) * 128],
                        rhs=ao2T[:, c2, tb * 512:(tb + 1) * 512],
                        start=(c2 == 0), stop=(c2 == 1),
                    )
                ev = evp.tile([128, 512], BF16, tag="ev")
                if fc % 2 == 0:
                    nc.vector.tensor_copy(ev[:], po[:])
                else:
                    nc.scalar.copy(ev[:], po[:])
                eng = nc.sync if fc % 2 == 0 else nc.scalar
                eng.dma_start(outT[fc, :, tb * 512:(tb + 1) * 512], ev[:])

            pth = singles.tile([128, 4, NT, QW], BF16)

            def weave(*lists):
                # round-robin over emission thunk lists (PE stays fed)
                lists = [list(l) for l in lists]
                while any(lists):
                    for l in lists:
                        if l:
                            l.pop(0)()

            def qkv_thunks(tb):
                out = []
                for m in range(4):
                    out.append(lambda m=m: qkv_group(tb, m))
                for half in range(2):
                    out.append(lambda half=half: v_group(tb, half))
                return out

            def s_thunks(jlo, jhi):
                return [lambda sq=sq, jb=jb: s_pair(sq, jb)
                        for jb in range(jlo, jhi) for sq in range(2)]

            def av_thunks(g):
                return [lambda sq=sq, hh=hh: av_block(sq, hh, g)
                        for sq in range(2) for hh in range(2)]

            def op_thunks(tb):
                return [lambda fc=fc: op_group(tb, fc) for fc in range(8)]

            # ---- pipelined emission ----
            load_x(1)
            for t in qkv_thunks(0):
                t()
            load_x(2)
            for t in qkv_thunks(1):
                t()
            load_x(3)
            weave(s_thunks(0, 4), qkv_thunks(2))
            weave(av_thunks(0), s_thunks(4, 6))
            weave(s_thunks(6, 8), qkv_thunks(3))
            weave(av_thunks(1), s_thunks(8, 10))
            weave(op_thunks(0), s_thunks(10, 12))
            weave(av_thunks(2), s_thunks(12, 14))
            weave(op_thunks(1), s_thunks(14, 16))
            weave(av_thunks(3), op_thunks(2))
            for t in op_thunks(3):
                t()
            if dbg:
                nc.sync.dma_start(d_qkvT[:], qkvT[:])
                nc.sync.dma_start(d_vnat[:], vnat[:])
                nc.sync.dma_start(d_pth[:], pth[:])
                nc.sync.dma_start(d_ao2T[:], ao2T[:])

    nc.compile()
    return nc


def _host_inputs(query, in_proj_weight, out_proj_weight):
    """Per-core input maps (numpy only)."""
    slopes = np.asarray(_get_slopes(H), np.float32)
    q32 = np.asarray(query, np.float32)
    w_in = np.asarray(in_proj_weight, np.float32)
    w_out = np.asarray(out_proj_weight, np.float32)

    # band+alibi bias tiles, shift-invariant per head: [h, jj, cc]
    jj = np.arange(128)[:, None]
    cc = np.arange(QW)[None, :]
    allowed = (cc >= jj) & (cc - jj <= W)
    in_maps = []
    for c in range(NCORES):
        b, hq = divmod(c, 4)
        heads = np.arange(4 * hq, 4 * hq + HL)
        rows = (heads[:, None] * HD + np.arange(HD)[None, :]).reshape(-1)  # 256
        wq = w_in[rows, :] * (1.0 / math.sqrt(HD))
        wk = w_in[E + rows, :]
        wv = w_in[2 * E + rows, :]
        w_loc = np.concatenate([wq, wk, wv], axis=0)          # [768, E]
        wqkv = np.ascontiguousarray(
            w_loc.T.reshape(8, 128, 768)).astype(ml_dtypes.bfloat16)

        xTc = np.ascontiguousarray(
            q32[b].T.reshape(8, 128, 4, 512).transpose(2, 0, 1, 3)
        ).astype(ml_dtypes.bfloat16)

        wo_loc = np.ascontiguousarray(
            w_out[:, rows].T.reshape(2, 128, E)).astype(ml_dtypes.bfloat16)

        biasd = np.empty((HL, 128, QW), ml_dtypes.bfloat16)
        for hl in range(HL):
            s = slopes[4 * hq + hl]
            eb = np.where(allowed, np.exp(-s * (cc - jj).astype(np.float64)), 0.0)
            biasd[hl] = eb.astype(ml_dtypes.bfloat16)

        in_maps.append({"xT": xTc, "wqkv": wqkv, "wo": wo_loc, "biasd": biasd})
    return in_maps


def _assemble(results):
    out = np.zeros((B, T, E), np.float32)
    for c in range(NCORES):
        b = c // 4
        part = np.asarray(results[c]["outT"]).astype(np.float32)  # [8,128,T]
        out[b] += part.reshape(E, T).T
    return out


def kernel(query, in_proj_weight, out_proj_weight, num_heads, sliding_window_size):
    assert int(num_heads) == H and int(sliding_window_size) == W
    assert query.shape == (B, T, E)
    if "nc" not in _CACHE:
        _CACHE["nc"] = _build()
    in_maps = _host_inputs(query, in_proj_weight, out_proj_weight)
    res = run_bass_kernel_spmd(_CACHE["nc"], in_maps, list(range(NCORES))).results
    return _assemble(res)


# revision 31
# speedup vs baseline: 1.0275x; 1.0275x over previous
"""Trainium2 Bass kernel for CachedMultiheadAttention (sliding-window + ALiBi).

Sharding: 8 cores = 2 batches x 4 head-quartets. Core c handles batch c//4 and
heads [4*(c%4), 4*(c%4)+4). Each core computes QKV projection for its heads,
banded attention (causal + 512 window + ALiBi), and a partial out-projection
over its heads' 256 embedding columns. Host sums the 4 partials per batch.

v2: all-bf16 dataflow, software-pipelined phases (QKV / attention / out-proj
interleaved so PE never drains), V produced directly in natural [t, d] layout
(x stationary, wv streaming; no PE transposes), single-instruction 640-wide
exp from a 2-bank PSUM tile, P*ebias multiplies on gpsimd, fast approximate
reciprocal for softmax denominators. Engine budget per core ~: PE 80us,
ACT ~60us, DVE ~60us, gpsimd ~50us.
"""
import math

import numpy as np
import ml_dtypes

import concourse.bass as bass
import concourse.tile as tile
from concourse import bacc, mybir
from concourse.bass_utils import run_bass_kernel_spmd

F32 = mybir.dt.float32
BF16 = mybir.dt.bfloat16

B, T, E, H, HD, W = 2, 2048, 1024, 16, 64, 512
NCORES = 8
HL = 4                # local heads per core
NT = T // 128         # 16 key blocks
QW = 640              # q-window per key block (128 + 512 sliding window)

_CACHE = {}


def _get_slopes(n):
    def p2(m):
        start = 2 ** (-(2 ** (-(math.log2(m) - 3))))
        return [start * start**i for i in range(m)]
    if math.log2(n) % 1 == 0:
        return p2(n)
    c = 2 ** math.floor(math.log2(n))
    return p2(c) + _get_slopes(2 * c)[0::2][: n - c]


def _build(dbg=False):
    nc = bacc.Bacc("TRN2", target_bir_lowering=False, debug=False, num_devices=NCORES)
    xT = nc.dram_tensor("xT", [4, 8, 128, 512], BF16, kind="ExternalInput").ap()
    wqkv = nc.dram_tensor("wqkv", [8, 128, 768], BF16, kind="ExternalInput").ap()
    wo = nc.dram_tensor("wo", [2, 128, E], BF16, kind="ExternalInput").ap()
    biasd = nc.dram_tensor("biasd", [HL, 128, QW], BF16, kind="ExternalInput").ap()
    outT = nc.dram_tensor("outT", [8, 128, T], BF16, kind="ExternalOutput").ap()
    if dbg:
        d_qkvT = nc.dram_tensor("d_qkvT", [128, 4, T], BF16, kind="ExternalOutput").ap()
        d_vnat = nc.dram_tensor("d_vnat", [128, NT, HL, HD + 1], BF16, kind="ExternalOutput").ap()
        d_pth = nc.dram_tensor("d_pth", [128, 4, NT, QW], BF16, kind="ExternalOutput").ap()
        d_ao2T = nc.dram_tensor("d_ao2T", [128, 2, T], BF16, kind="ExternalOutput").ap()
        d_aoraw = nc.dram_tensor("d_aoraw", [4, 65, 512], F32, kind="ExternalOutput").ap()
        d_rec = nc.dram_tensor("d_rec", [4, 1, 512], F32, kind="ExternalOutput").ap()
        d_bc = nc.dram_tensor("d_bc", [4, 64, 512], F32, kind="ExternalOutput").ap()

    with tile.TileContext(nc) as tc:
        with (
            tc.tile_pool(name="singles", bufs=1) as singles,
            tc.tile_pool(name="xp", bufs=3) as xp,
            tc.tile_pool(name="prawp", bufs=3) as prawp,
            tc.tile_pool(name="smallp", bufs=3) as smallp,
            tc.tile_pool(name="evp", bufs=4) as evp,
            tc.tile_pool(name="mm", bufs=2, space="PSUM") as mmp,
            tc.tile_pool(name="av", bufs=2, space="PSUM") as avp,
            tc.tile_pool(name="sp", bufs=2, space="PSUM") as spp,
        ):
            wqkv_sb = singles.tile([128, 8, 768], BF16)
            wo_sb = singles.tile([128, 2, E], BF16)
            bias_sb = singles.tile([128, HL, QW], BF16)
            qkvT = singles.tile([128, 4, T], BF16)   # slots: Qp0 Qp1 Kp0 Kp1
            vnat = singles.tile([128, NT, HL, HD + 1], BF16)
            ao2T = singles.tile([128, 2, T], BF16)   # normalized AO^T (bf16)

            nc.gpsimd.memset(vnat[:], 1.0)           # ones col at [..., 64]

            xcs = {}
            xT_r = xT.rearrange("b c p t -> b p c t")

            def load_x(tb):
                xc = xp.tile([128, 8, 512], BF16, tag="xc")
                xcs[tb] = xc
                for ec in range(8):
                    eng = nc.sync if ec % 2 == 0 else nc.scalar
                    eng.dma_start(xc[:, ec, :], xT[tb, ec])

            # PE warmup: dummy matmuls on zeros keep HAM busy while the
            # initial DMAs land, so real QKV work starts at full clock
            wup = singles.tile([128, 640], BF16)
            nc.vector.memset(wup[:], 0.0)
            wup_ps = mmp.tile([128, 512], F32, tag="mm512")
            for _ in range(24):
                nc.tensor.matmul(
                    wup_ps[:], lhsT=wup[:, 0:128], rhs=wup[:, 0:512],
                    start=True, stop=True,
                )

            # loads in descending priority on the two HWDGE queues: the queue
            # order is the DGE service order, so first-needed bytes go first.
            xc0 = xp.tile([128, 8, 512], BF16, tag="xc")
            xcs[0] = xc0
            wq_r = wqkv.rearrange("c p m -> p c m")
            for ec in range(8):
                e1, e2 = (nc.sync, nc.scalar) if ec % 2 == 0 else (nc.scalar, nc.sync)
                e1.dma_start(xc0[:, ec, :], xT[0, ec])
                e2.dma_start(wqkv_sb[:, ec, 0:512], wq_r[:, ec, 0:512])
            for ec in range(8):
                eng = nc.sync if ec % 2 == 0 else nc.scalar
                eng.dma_start(wqkv_sb[:, ec, 512:768], wq_r[:, ec, 512:768])

            def qkv_group(tb, m):
                # Q/K projection chunk m (0..3): transposed layout [d_local, t]
                xc = xcs[tb]
                pt = mmp.tile([128, 512], F32, tag="mm512")
                for ec in range(8):
                    nc.tensor.matmul(
                        pt[:],
                        lhsT=wqkv_sb[:, ec, m * 128:(m + 1) * 128],
                        rhs=xc[:, ec, :],
                        start=(ec == 0), stop=(ec == 7),
                    )
                nc.vector.tensor_copy(qkvT[:, m, tb * 512:(tb + 1) * 512], pt[:])

            def v_group(tb, half):
                # V in natural layout: x chunk stationary, wv streaming.
                # Two t-chunks share one PSUM tile / one evac.
                xc = xcs[tb]
                tc0 = tb * 4 + 2 * half
                pv = mmp.tile([128, 512], F32, tag="mm512")
                for sub in range(2):
                    tcq = 2 * half + sub
                    for ec in range(8):
                        nc.tensor.matmul(
                            pv[:, sub * 256:(sub + 1) * 256],
                            lhsT=xc[:, ec, tcq * 128:(tcq + 1) * 128],
                            rhs=wqkv_sb[:, ec, 512:768],
                            start=(ec == 0), stop=(ec == 7),
                            skip_group_check=True,
                        )
                nc.scalar.copy(
                    vnat[:, tc0:tc0 + 2, 0:4, 0:HD],
                    pv[:].rearrange("p (s h d) -> p s h d", s=2, h=4),
                )

            def s_pair(sq, jb):
                # S^T[j, q] for both heads of pair sq, q in [jb*128, jb*128+qw).
                # hh=0 on PE rows 0:64, hh=1 on rows 64:128 -> the two K=64
                # matmul streams run concurrently on the array.
                qw = min(QW, T - jb * 128)
                w0 = min(qw, 512)
                sts = []
                for hh in range(2):
                    r0 = hh * 64
                    st = spp.tile([128, QW], F32, tag="s640")
                    sts.append(st)
                    nc.tensor.matmul(
                        st[:, 0:w0],
                        lhsT=qkvT[r0:r0 + 64, 2 + sq, jb * 128:(jb + 1) * 128],
                        rhs=qkvT[r0:r0 + 64, sq, jb * 128:jb * 128 + w0],
                        start=True, stop=True,
                    )
                for hh in range(2):
                    if qw > 512:
                        r0 = hh * 64
                        nc.tensor.matmul(
                            sts[hh][:, 512:qw],
                            lhsT=qkvT[r0:r0 + 64, 2 + sq, jb * 128:(jb + 1) * 128],
                            rhs=qkvT[r0:r0 + 64, sq, jb * 128 + 512:jb * 128 + qw],
                            start=True, stop=True,
                        )
                for hh in range(2):
                    h = 2 * sq + hh
                    praw = prawp.tile([128, QW], BF16, tag="praw")
                    nc.scalar.activation(
                        out=praw[:, 0:qw], in_=sts[hh][:, 0:qw],
                        func=mybir.ActivationFunctionType.Exp,
                    )
                    nc.vector.tensor_tensor(
                        out=pth[:, h, jb, 0:qw], in0=praw[:, 0:qw],
                        in1=bias_sb[:, h, 0:qw], op=mybir.AluOpType.mult,
                    )

            def av_block(sq, hh, g):
                h = 2 * sq + hh
                r0 = hh * 64
                ao = avp.tile([128, 512], F32, tag="ao")
                jbs = [4 * g] + [jb for jb in range(max(0, 4 * g - 4), 4 * g + 4)
                                 if jb != 4 * g]
                for i, jb in enumerate(jbs):
                    qb_lo = max(4 * g, jb)
                    qb_hi = min(4 * g + 3, jb + 4)
                    wdt = (qb_hi - qb_lo + 1) * 128
                    ao_off = (qb_lo - 4 * g) * 128
                    p_off = (qb_lo - jb) * 128
                    nc.tensor.matmul(
                        ao[0:65, ao_off:ao_off + wdt],
                        lhsT=vnat[:, jb, hh + 2 * sq, :],
                        rhs=pth[:, 2 * sq + hh, jb, p_off:p_off + wdt],
                        start=(i == 0), stop=(i == len(jbs) - 1),
                        skip_group_check=True,
                    )
                rs0 = smallp.tile([1, 512], F32, tag="rs0")
                nc.scalar.copy(rs0[:], ao[64:65, :])
                rec = smallp.tile([1, 512], F32, tag="rec")
                nc.vector.reciprocal_approx_fast(rec[:], rs0[:])
                bc = smallp.tile([64, 512], F32, tag="bc")
                nc.gpsimd.partition_broadcast(bc[:], rec[:])
                if dbg and g == 0:
                    di = 2 * sq + hh
                    draw = evp.tile([65, 512], F32, tag="draw")
                    nc.vector.tensor_copy(draw[:], ao[0:65, :])
                    nc.sync.dma_start(d_aoraw[di], draw[:])
                    nc.sync.dma_start(d_rec[di], rec[:])
                    nc.sync.dma_start(d_bc[di], bc[:])
                nc.vector.tensor_tensor(
                    out=ao2T[r0:r0 + 64, sq, g * 512:(g + 1) * 512],
                    in0=ao[0:64, :], in1=bc[:], op=mybir.AluOpType.mult,
                )

            def op_group(tb, fc):
                po = mmp.tile([128, 512], F32, tag="mm512")
                for c2 in range(2):
                    nc.tensor.matmul(
                        po[:],
                        lhsT=wo_sb[:, c2, fc * 128:(fc + 1) * 128],
                        rhs=ao2T[:, c2, tb * 512:(tb + 1) * 512],
                        start=(c2 == 0), stop=(c2 == 1),
                    )
                ev = evp.tile([128, 512], BF16, tag="ev")
                if fc % 2 == 0:
                    nc.vector.tensor_copy(ev[:], po[:])
                else:
                    nc.scalar.copy(ev[:], po[:])
                eng = nc.sync if fc % 2 == 0 else nc.scalar
                eng.dma_start(outT[fc, :, tb * 512:(tb + 1) * 512], ev[:])

            pth = singles.tile([128, 4, NT, QW], BF16)

            def weave(*lists):
                # round-robin over emission thunk lists (PE stays fed)
                lists = [list(l) for l in lists]
                while any(lists):
                    for l in lists:
                        if l:
                            l.pop(0)()

            def qkv_thunks(tb):
                out = []
                for m in range(4):
                    out.append(lambda m=m: qkv_group(tb, m))
                for half in range(2):
                    out.append(lambda half=half: v_group(tb, half))
                return out

            def s_thunks(jlo, jhi):
                return [lambda sq=sq, jb=jb: s_pair(sq, jb)
                        for jb in range(jlo, jhi) for sq in range(2)]

            def av_thunks(g):
                return [lambda sq=sq, hh=hh: av_block(sq, hh, g)
                        for sq in range(2) for hh in range(2)]

            def op_thunks(tb):
                return [lambda fc=fc: op_group(tb, fc) for fc in range(8)]

            # ---- pipelined emission ----
            load_x(1)
            nc.sync.dma_start(bias_sb[:], biasd.rearrange("h p c -> p h c"))
            nc.scalar.dma_start(wo_sb[:], wo.rearrange("c p f -> p c f"))
            for t in qkv_thunks(0):
                t()
            load_x(2)
            for t in qkv_thunks(1):
                t()
            load_x(3)
            weave(s_thunks(0, 4), qkv_thunks(2))
            weave(av_thunks(0), s_thunks(4, 6))
            weave(s_thunks(6, 8), qkv_thunks(3))
            weave(av_thunks(1), s_thunks(8, 10))
            weave(op_thunks(0), s_thunks(10, 12))
            weave(av_thunks(2), s_thunks(12, 14))
            weave(op_thunks(1), s_thunks(14, 16))
            weave(av_thunks(3), op_thunks(2))
            for t in op_thunks(3):
                t()
            if dbg:
                nc.sync.dma_start(d_qkvT[:], qkvT[:])
                nc.sync.dma_start(d_vnat[:], vnat[:])
                nc.sync.dma_start(d_pth[:], pth[:])
                nc.sync.dma_start(d_ao2T[:], ao2T[:])

    nc.compile()
    return nc


def _host_inputs(query, in_proj_weight, out_proj_weight):
    """Per-core input maps (numpy only)."""
    slopes = np.asarray(_get_slopes(H), np.float32)
    q32 = np.asarray(query, np.float32)
    w_in = np.asarray(in_proj_weight, np.float32)
    w_out = np.asarray(out_proj_weight, np.float32)

    # band+alibi bias tiles, shift-invariant per head: [h, jj, cc]
    jj = np.arange(128)[:, None]
    cc = np.arange(QW)[None, :]
    allowed = (cc >= jj) & (cc - jj <= W)
    in_maps = []
    for c in range(NCORES):
        b, hq = divmod(c, 4)
        heads = np.arange(4 * hq, 4 * hq + HL)
        rows = (heads[:, None] * HD + np.arange(HD)[None, :]).reshape(-1)  # 256
        wq = w_in[rows, :] * (1.0 / math.sqrt(HD))
        wk = w_in[E + rows, :]
        wv = w_in[2 * E + rows, :]
        w_loc = np.concatenate([wq, wk, wv], axis=0)          # [768, E]
        wqkv = np.ascontiguousarray(
            w_loc.T.reshape(8, 128, 768)).astype(ml_dtypes.bfloat16)

        xTc = np.ascontiguousarray(
            q32[b].T.reshape(8, 128, 4, 512).transpose(2, 0, 1, 3)
        ).astype(ml_dtypes.bfloat16)

        wo_loc = np.ascontiguousarray(
            w_out[:, rows].T.reshape(2, 128, E)).astype(ml_dtypes.bfloat16)

        biasd = np.empty((HL, 128, QW), ml_dtypes.bfloat16)
        for hl in range(HL):
            s = slopes[4 * hq + hl]
            eb = np.where(allowed, np.exp(-s * (cc - jj).astype(np.float64)), 0.0)
            biasd[hl] = eb.astype(ml_dtypes.bfloat16)

        in_maps.append({"xT": xTc, "wqkv": wqkv, "wo": wo_loc, "biasd": biasd})
    return in_maps


def _assemble(results):
    out = np.zeros((B, T, E), np.float32)
    for c in range(NCORES):
        b = c // 4
        part = np.asarray(results[c]["outT"]).astype(np.float32)  # [8,128,T]
        out[b] += part.reshape(E, T).T
    return out


def kernel(query, in_proj_weight, out_proj_weight, num_heads, sliding_window_size):
    assert int(num_heads) == H and int(sliding_window_size) == W
    assert query.shape == (B, T, E)
    if "nc" not in _CACHE:
        _CACHE["nc"] = _build()
    in_maps = _host_inputs(query, in_proj_weight, out_proj_weight)
    res = run_bass_kernel_spmd(_CACHE["nc"], in_maps, list(range(NCORES))).results
    return _assemble(res)
